# revision 1
# baseline (speedup 1.0000x reference)
"""Trainium2 kernel for nn_ApplyPolicyMap (lc0 policy-map apply).

out = reshape(x, [B, 5120]) @ fc1, where fc1 is a fixed 0/1 selection
matrix: every one of the 1858 output columns selects exactly one of the
5120 input features.  So the matmul is a feature gather:
    out[b, m] = x_flat[b, src_idx[m]],   src_idx = argmax(fc1, axis=0)

Distribution: shard x along the FEATURE dim across the 8 cores (640
features each).  Core i computes the output moves sourced from its
feature slice.  On-device per core:
  load:   one striped SWDGE DMA per 2048-row group, casting f32 -> bf16
          in flight; partition p holds 16 consecutive batch rows (big
          contiguous descriptors).
  pass 1: transpose each 128-feature chunk with the PE transpose mode
          (stationary = x tile, stream identity) -> features on
          partitions, bf16 PSUM.
  pass 2: tiny one-hot selection matmuls (stationary = transposed x
          slice, stream the per-core selection matrix derived from fc1
          on host) -> gathered output directly in batch-major layout.
  store:  bf16 output tile (the gathered values are exactly bf16), one
          striped DMA per group.
Host reassembles the full [B, 1858] f32 output by placing each core's
move columns at their final positions.  Total error = bf16 quantization
of x only (~1.7e-3 L2 relative).
"""

import os
from contextlib import ExitStack

import ml_dtypes
import numpy as np

import concourse.bass as bass
import concourse.tile as tile
from concourse import bacc, mybir
from concourse.bass_utils import run_bass_kernel_spmd

N_CORES = 8
B = 16384
PLANES = 80
FLAT = PLANES * 64          # 5120
N_MOVES = 1858
F_PER_CORE = FLAT // N_CORES  # 640
N_CHUNKS = F_PER_CORE // 128  # 5
# padded move capacity per 128-feature chunk slot (max across cores for the
# fixed seed-0 policy map; recomputed at runtime if the map ever differs)
DEFAULT_CAPS = (55, 58, 56, 56, 61)
B_TILE = 128
J = 16                        # batch rows per partition per group
B_GROUP = 128 * J             # 2048
N_GROUPS = B // B_GROUP       # 8

F32 = mybir.dt.float32
BF16 = mybir.dt.bfloat16

# Set by test harness to capture a neuron profile.
TRACE = bool(int(os.environ.get("KERNEL_TRACE", "0")))
TRACE_DIR = os.environ.get("KERNEL_TRACE_DIR") or None
LAST_RESULTS = None  # BassKernelResults of the most recent run (for profiling)


def _build_bass(caps):
    offs = [0]
    for c in caps:
        offs.append(offs[-1] + c)
    out_cols = offs[-1]
    nc = bacc.Bacc("TRN2", target_bir_lowering=False, debug=False)

    x = nc.dram_tensor("x", [B, F_PER_CORE], F32, kind="ExternalInput").ap()
    sel = nc.dram_tensor("sel", [128, out_cols], BF16, kind="ExternalInput").ap()
    ident = nc.dram_tensor("ident", [128, 128], BF16, kind="ExternalInput").ap()
    out = nc.dram_tensor("out", [B, out_cols], BF16, kind="ExternalOutput").ap()

    with tile.TileContext(nc) as tc, ExitStack() as ctx:
        const_pool = ctx.enter_context(tc.tile_pool(name="const", bufs=1))
        x_pool = ctx.enter_context(tc.tile_pool(name="xin", bufs=4))
        xT_pool = ctx.enter_context(tc.tile_pool(name="xT", bufs=9))
        o_pool = ctx.enter_context(tc.tile_pool(name="obuf", bufs=4))
        psum1 = ctx.enter_context(tc.tile_pool(name="psum1", bufs=2, space="PSUM"))
        psum2 = ctx.enter_context(tc.tile_pool(name="psum2", bufs=4, space="PSUM"))

        sel_t = const_pool.tile([128, out_cols], BF16)
        nc.sync.dma_start(sel_t[:], sel[:])
        id_t = const_pool.tile([128, 128], BF16)
        nc.sync.dma_start(id_t[:], ident[:])

        for g in range(N_GROUPS - 1):
            # One striped load per group: partition p holds batch rows
            # [r+J*p, r+J*p+J) -> 40KB-contiguous DRAM descriptors (few
            # descriptors keeps SWDGE ring traffic off the hot AXI ports).
            # SWDGE casts f32 -> bf16 in flight.
            r = g * B_GROUP
            xt = x_pool.tile([128, J, F_PER_CORE], BF16)
            nc.gpsimd.dma_start(
                xt[:], x[r : r + B_GROUP, :].rearrange("(p j) f -> p j f", j=J)
            )

            # pass 1: transpose every chunk -> features on partitions.
            # j indexes the b-stripe (b = r + J*p + j).
            xTs = []
            for c in range(N_CHUNKS):
                p1 = psum1.tile([128, B_GROUP], BF16)
                for j in range(J):
                    nc.tensor.matmul(
                        p1[:, 128 * j : 128 * (j + 1)],
                        lhsT=xt[:, j, 128 * c : 128 * (c + 1)],
                        rhs=id_t[:],
                        start=True,
                        stop=True,
                        is_transpose=True,
                    )
                xTc = xT_pool.tile([128, B_GROUP], BF16, name=f"xT_{g}_{c}", tag="xT")
                nc.vector.tensor_copy(xTc[:], p1[:])
                xTs.append(xTc)

            # pass 2: gather straight into final batch-major layout:
            # psum_j[p, m] = out value for batch row r + J*p + j
            ot = o_pool.tile([128, J, out_cols], BF16)
            out_v = out[r : r + B_GROUP, :].rearrange("(p j) m -> p j m", j=J)
            for j in range(J):
                p2 = psum2.tile([128, out_cols], F32, name=f"p2_{g}_{j}", tag="p2")
                for c in range(N_CHUNKS):
                    nc.tensor.matmul(
                        p2[:, offs[c] : offs[c + 1]],
                        lhsT=xTs[c][:, 128 * j : 128 * (j + 1)],
                        rhs=sel_t[:, offs[c] : offs[c + 1]],
                        start=True,
                        stop=True,
                    )
                if j % 2 == 0:
                    nc.vector.tensor_copy(ot[:, j, :], p2[:])
                else:
                    nc.scalar.copy(ot[:, j, :], p2[:])
            nc.sync.dma_start(out_v[:], ot[:])

        # Last group: quarter-granular sub-loads and compute so the tail
        # overlaps the final input stream (adds only ~3 extra SWDGE DMAs).
        JQ = J // 4
        r = (N_GROUPS - 1) * B_GROUP
        src = x[r : r + B_GROUP, :].rearrange("(p j) f -> p j f", j=J)
        out_v = out[r : r + B_GROUP, :].rearrange("(p j) m -> p j m", j=J)
        ot = o_pool.tile([128, J, out_cols], BF16, name="otL", tag="ot")
        xqs = []
        for q in range(4):
            xq = x_pool.tile([128, JQ, F_PER_CORE], BF16, name=f"xqL_{q}", tag="xq")
            nc.gpsimd.dma_start(xq[:], src[:, JQ * q : JQ * (q + 1), :])
            xqs.append(xq)
        for q in range(4):
            xTs = []
            for c in range(N_CHUNKS):
                p1 = psum1.tile([128, B_GROUP], BF16, name=f"p1L_{q}_{c}", tag="p1")
                for jj in range(JQ):
                    nc.tensor.matmul(
                        p1[:, 128 * jj : 128 * (jj + 1)],
                        lhsT=xqs[q][:, jj, 128 * c : 128 * (c + 1)],
                        rhs=id_t[:],
                        start=True,
                        stop=True,
                        is_transpose=True,
                    )
                xTc = xT_pool.tile([128, B_GROUP], BF16, name=f"xTL_{q}_{c}", tag="xT")
                nc.vector.tensor_copy(xTc[:, : 128 * JQ], p1[:, : 128 * JQ])
                xTs.append(xTc)
            for jj in range(JQ):
                j = JQ * q + jj
                p2 = psum2.tile([128, out_cols], F32, name=f"p2L_{j}", tag="p2")
                for c in range(N_CHUNKS):
                    nc.tensor.matmul(
                        p2[:, offs[c] : offs[c + 1]],
                        lhsT=xTs[c][:, 128 * jj : 128 * (jj + 1)],
                        rhs=sel_t[:, offs[c] : offs[c + 1]],
                        start=True,
                        stop=True,
                    )
                if j % 2 == 0:
                    nc.vector.tensor_copy(ot[:, j, :], p2[:])
                else:
                    nc.scalar.copy(ot[:, j, :], p2[:])
            nc.sync.dma_start(
                out_v[:, JQ * q : JQ * (q + 1), :], ot[:, JQ * q : JQ * (q + 1), :]
            )

    nc.compile()
    return nc


_NC_CACHE = {}


def _get_nc(caps):
    caps = tuple(caps)
    if caps not in _NC_CACHE:
        _NC_CACHE[caps] = _build_bass(caps)
    return _NC_CACHE[caps]


def _make_policy_map_idx():
    # Deterministic stand-in policy map from the reference (seed 0).
    rng = np.random.RandomState(0)
    return rng.permutation(FLAT)[:N_MOVES].astype(np.int64)


def kernel(x, fc1=None):
    global LAST_RESULTS
    x = np.asarray(x, dtype=np.float32)
    x_flat = np.ascontiguousarray(x.reshape(B, FLAT))
    if fc1 is not None:
        src_idx = np.argmax(np.asarray(fc1), axis=0).astype(np.int64)
    else:
        src_idx = _make_policy_map_idx()

    ident = np.eye(128, dtype=ml_dtypes.bfloat16)

    # per-chunk-slot capacities (shared across cores; SPMD needs one shape)
    chunk_of = src_idx // 128          # 0..39
    slot_of = chunk_of % N_CHUNKS      # chunk slot within its core
    core_of = src_idx // F_PER_CORE
    need = np.zeros((N_CORES, N_CHUNKS), dtype=np.int64)
    np.add.at(need, (core_of, slot_of), 1)
    need_caps = need.max(axis=0)
    if np.all(need_caps <= np.array(DEFAULT_CAPS)):
        caps = DEFAULT_CAPS
    else:
        caps = tuple(int(v) for v in need_caps)
    offs = [0]
    for c in caps:
        offs.append(offs[-1] + c)
    out_cols = offs[-1]

    in_maps = []
    placement = []  # (final move cols, padded cols) per core
    for i in range(N_CORES):
        f0 = i * F_PER_CORE
        sel_i = np.zeros((128, out_cols), dtype=np.float32)
        fcols, pcols = [], []
        for j in range(N_CHUNKS):
            lo = f0 + 128 * j
            moves = np.where((src_idx >= lo) & (src_idx < lo + 128))[0]
            for k, m in enumerate(moves):
                sel_i[src_idx[m] - lo, offs[j] + k] = 1.0
                fcols.append(m)
                pcols.append(offs[j] + k)
        placement.append((np.array(fcols), np.array(pcols)))
        x_shard = np.ascontiguousarray(x_flat[:, f0 : f0 + F_PER_CORE])
        in_maps.append(
            {"x": x_shard, "sel": sel_i.astype(ml_dtypes.bfloat16), "ident": ident}
        )

    nc = _get_nc(caps)
    res = None
    for attempt in range(3):
        try:
            res = run_bass_kernel_spmd(
                nc, in_maps, core_ids=list(range(N_CORES)), trace=TRACE, tmpdir=TRACE_DIR
            )
            break
        except Exception:
            # Rare transient NRT_EXEC_UNIT_UNRECOVERABLE on first exec of a
            # freshly compiled NEFF; retry.
            if attempt == 2:
                raise
            import time as _time

            _time.sleep(2.0)
    LAST_RESULTS = res

    out_full = np.empty((B, N_MOVES), dtype=np.float32)
    for i in range(N_CORES):
        fcols, pcols = placement[i]
        out_full[:, fcols] = res.results[i]["out"][:, pcols].astype(np.float32)
    return out_full

